# revision 64
# baseline (speedup 1.0000x reference)
"""Trainium2 Bass kernel for nn_DSQGAttentionN (banded sparse attention).

Sharding: 8 cores = 2 batches x 4 head-groups (4 heads each), all-fp16
matmul pipeline with fp32 PSUM accumulation.

Per-core device program (identical program across cores, data differs):
  A: qkT/kT [dh, tok] via matmul with host-permuted Wqkv columns
     (q columns pre-scaled by HD^-0.5); bias add on DVE evacuation.
  B: V token-major [tok, dv]; v-bias folded in via a ones-row matmul;
     per-head 128-col blocks [ones, 63 zero pad, v] so the AV matmul puts
     the softmax denominator on partition 0 and products on 64:127.
  C: gate projection in fp8e4 DoubleRow (4x PE throughput); sigmoid
     computed as 0.5*tanh(z/2)+0.5 on Act (Tanh shares the Exp act table,
     avoiding table reloads) + affine fixup.
  D: banded attention: per (head, 128-query block) only relative key
     chunks {0,1,2,3,4,6,8,12} contain any of the 44 taps. Transposed
     score tiles S^T[j,i] -> exp on Act -> multiply by exp(pos_bias) tap
     mask on DVE -> AV+denominator matmul per chunk. Software-pipelined
     several query-blocks ahead; the next chunk's projections and the
     previous group's output projection interleave as PE filler.
  E: per query-block-group: reciprocal of denominator row (DVE, partition
     0), partition-broadcast (Pool), normalize + gate multiply on DVE
     (even heads lifted 64:128 -> 0:64 via a small identity matmul);
     output projection -> partial y [2048, 1024] f16.
Host: sums the 4 head-group partials per batch, adds bout.

Hardware-validated constraints baked in: GPSIMD/Pool cannot access PSUM;
custom DVE ops (reciprocal_approx_fast) and partition_broadcast only work
from base partition 0; TensorTensor operands must share a start partition.
"""

import numpy as np

import concourse.bass as bass
import concourse.mybir as mybir
import concourse.tile as tile
from concourse import bacc
from concourse.bass_utils import run_bass_kernel_spmd

F32 = mybir.dt.float32
F16 = mybir.dt.float16
F8 = mybir.dt.float8e4

B, N, D, H = 2, 2048, 1024, 16
HD = D // H
HG = 4            # heads per core
NB = N // 128     # 16 query blocks
G = [0, 1, 2, 3, 4, 6, 8, 12]   # relative key chunks that contain taps
OFFSETS = sorted(set(range(0, 33)) | {48, 64, 96, 128, 192, 256, 384, 512, 768, 1024, 1536})


def build_nc():
    nc = bacc.Bacc("TRN2", target_bir_lowering=False, debug=False)

    xT = nc.dram_tensor("xT", [128, 8, N], F16, kind="ExternalInput")
    wqk = nc.dram_tensor("wqk", [128, 8, 512], F16, kind="ExternalInput")
    wv = nc.dram_tensor("wv", [128, 8, 256], F16, kind="ExternalInput")
    xT8 = nc.dram_tensor("xT8", [128, 4, 2, N], F8, kind="ExternalInput")
    wg8 = nc.dram_tensor("wg8", [128, 4, 2, 256], F8, kind="ExternalInput")
    wo = nc.dram_tensor("wo", [128, 2, D], F16, kind="ExternalInput")
    maskt = nc.dram_tensor("maskt", [128, HG, len(G), 128], F16, kind="ExternalInput")
    bqk2 = nc.dram_tensor("bqk2", [128, 4], F32, kind="ExternalInput")
    bg2 = nc.dram_tensor("bg2", [128, 2], F32, kind="ExternalInput")
    bvrow = nc.dram_tensor("bvrow", [1, 256], F16, kind="ExternalInput")
    idhi = nc.dram_tensor("idhi", [128, 64], F16, kind="ExternalInput")
    y = nc.dram_tensor("y", [N, D], F16, kind="ExternalOutput")

    with tile.TileContext(nc) as tc:
        with tc.tile_pool(name="persist", bufs=1) as persist:
            qkT = persist.tile([128, 4, N], F16)         # [part, (q01,q23,k01,k23), tok]
            # per-head 128-col block: [ones, 63 x zero pad, v0..v63] so the
            # AV matmul lands the denominator on partition 0 (custom DVE
            # reciprocal + partition_broadcast only work from partition 0)
            # and products on partitions 64:127
            vsb = persist.tile([128, NB, HG * 128], F16)
            gateT = persist.tile([128, 2, N], F16)
            wo_sb = persist.tile([128, 2, D], F16)
            maskt_sb = persist.tile([128, HG, len(G), 128], F16)
            bqk2_sb = persist.tile([128, 4], F32)
            bg2_sb = persist.tile([128, 2], F32)
            bvrow_sb = persist.tile([1, 256], F16)
            ones1 = persist.tile([1, 128], F16)
            idhi_sb = persist.tile([128, 64], F16)
            fgfinal = persist.tile([128, 2, N], F16)

            # parallel DGE queues: critical-path loads (wqk, x) on SP;
            # the rest on the Act queue in first-use order.
            nc.scalar.dma_start(out=bqk2_sb, in_=bqk2.ap())
            nc.scalar.dma_start(out=bg2_sb, in_=bg2.ap())
            nc.scalar.dma_start(out=bvrow_sb, in_=bvrow.ap())
            nc.vector.memset(ones1, 1.0)
            nc.scalar.dma_start(out=idhi_sb, in_=idhi.ap())
            nc.vector.memset(
                vsb.rearrange("p b (h u) -> p b h u", u=128)[:, :, :, 0:64], 0.0)
            nc.vector.memset(
                vsb.rearrange("p b (h u) -> p b h u", u=128)[:, :, :, 0], 1.0)

            with (
                tc.tile_pool(name="psproj", bufs=2, space="PSUM") as psproj,
                tc.tile_pool(name="psst", bufs=2, space="PSUM") as psst,
                tc.tile_pool(name="psav", bufs=2, space="PSUM") as psav,
                tc.tile_pool(name="dpool", bufs=4) as dpool,
                tc.tile_pool(name="epool", bufs=4) as epool,
                tc.tile_pool(name="ypool", bufs=4) as ypool,
            ):
                load = tc.alloc_tile_pool(name="load", bufs=1)
                xT_sb = load.tile([128, 8, N], F16)
                wqk_sb = load.tile([128, 8, 512], F16)
                wv_sb = load.tile([128, 8, 256], F16)
                xT8_sb = load.tile([128, 4, 2, N], F8)
                wg8_sb = load.tile([128, 4, 2, 256], F8)
                def dma_x(c):
                    nc.sync.dma_start(
                        out=xT_sb[:, :, c * 256:(c + 1) * 256],
                        in_=xT.ap()[:, :, c * 256:(c + 1) * 256])

                dma_x(0)
                for _gi in range(0, 4):
                    nc.sync.dma_start(
                        out=wqk_sb[:, :, _gi * 128:(_gi + 1) * 128],
                        in_=wqk.ap()[:, :, _gi * 128:(_gi + 1) * 128])
                dma_x(1)
                nc.sync.dma_start(out=wv_sb, in_=wv.ap())
                dma_x(2)
                dma_x(3)
                nc.sync.dma_start(out=wg8_sb, in_=wg8.ap())
                nc.sync.dma_start(
                    out=xT8_sb[:, :, :, 0:512], in_=xT8.ap()[:, :, :, 0:512])
                nc.sync.dma_start(out=maskt_sb, in_=maskt.ap())
                dma_x(4)
                nc.sync.dma_start(
                    out=xT8_sb[:, :, :, 512:1024], in_=xT8.ap()[:, :, :, 512:1024])
                dma_x(5)
                dma_x(6)
                nc.sync.dma_start(
                    out=xT8_sb[:, :, :, 1024:2048], in_=xT8.ap()[:, :, :, 1024:2048])
                dma_x(7)
                nc.sync.dma_start(out=wo_sb, in_=wo.ap())

                def do_A(gi, nt):
                    # q/k projection group gi -> qkT[:, gi, :] (q pre-scaled);
                    # two 256-col halves (separate x chunks) into one PSUM
                    # tile, single 512-col evacuation
                    psw = psproj.tile([128, 512], F32, tag="proj")
                    c0 = nt * 512
                    if nt == 0:
                        for half in range(2):
                            for kc in range(8):
                                nc.tensor.matmul(
                                    psw[:, half * 256:(half + 1) * 256],
                                    lhsT=wqk_sb[:, kc, gi * 128:(gi + 1) * 128],
                                    rhs=xT_sb[:, kc, c0 + half * 256:c0 + (half + 1) * 256],
                                    start=(kc == 0), stop=(kc == 7),
                                    skip_group_check=True,
                                )
                    else:
                        for kc in range(8):
                            nc.tensor.matmul(
                                psw,
                                lhsT=wqk_sb[:, kc, gi * 128:(gi + 1) * 128],
                                rhs=xT_sb[:, kc, c0:c0 + 512],
                                start=(kc == 0), stop=(kc == 7),
                            )
                    if gi % 2 == 0:
                        nc.vector.tensor_scalar_add(
                            qkT[:, gi, c0:c0 + 512], psw,
                            bqk2_sb[:, gi:gi + 1])
                    else:
                        nc.scalar.activation(
                            qkT[:, gi, c0:c0 + 512], psw,
                            mybir.ActivationFunctionType.Identity,
                            bias=bqk2_sb[:, gi:gi + 1])

                def do_B(tci):
                    psv = psproj.tile([128, 512], F32, tag="proj")
                    for kc in range(8):
                        nc.tensor.matmul(
                            psv[:, 0:256],
                            lhsT=xT_sb[:, kc, tci * 128:(tci + 1) * 128],
                            rhs=wv_sb[:, kc, :],
                            start=(kc == 0), stop=False,
                        )
                    nc.tensor.matmul(
                        psv[:, 0:256], lhsT=ones1, rhs=bvrow_sb,
                        start=False, stop=True,
                    )
                    # GPSIMD cannot touch PSUM: evacuate on DVE/Act
                    vdst = vsb[:, tci, :].rearrange(
                        "p (h u) -> p h u", u=128)[:, :, 64:128]
                    psrc = psv[:, 0:256].rearrange("p (h u) -> p h u", u=64)
                    if tci % 2 == 0:
                        nc.vector.tensor_copy(vdst, psrc)
                    else:
                        nc.scalar.copy(vdst, psrc)

                def do_C(gi2, nt):
                    # gate = sigmoid(z) = 0.5*tanh(z/2)+0.5; Tanh shares the
                    # Exp act table so no table reloads; fixup on DVE.
                    psg = psproj.tile([128, 512], F32, tag="proj")
                    for kc2 in range(4):
                        nc.tensor.matmul(
                            psg,
                            lhsT=wg8_sb[:, kc2, :, gi2 * 128:(gi2 + 1) * 128],
                            rhs=xT8_sb[:, kc2, :, nt * 512:(nt + 1) * 512],
                            start=(kc2 == 0), stop=(kc2 == 3),
                            perf_mode=mybir.MatmulPerfMode.DoubleRow,
                        )
                    gslice = gateT[:, gi2, nt * 512:(nt + 1) * 512]
                    nc.scalar.activation(
                        gslice, psg,
                        mybir.ActivationFunctionType.Tanh,
                        bias=bg2_sb[:, gi2:gi2 + 1], scale=0.5,
                    )
                    nc.gpsimd.tensor_scalar(
                        gslice, gslice, 0.5, 0.5,
                        op0=mybir.AluOpType.mult, op1=mybir.AluOpType.add)

                def emit_scores(h, qb):
                    pq = 64 * (h % 2)
                    pg = h // 2
                    gs = [g for g in G if qb - g >= 0]  # prefix of G
                    ngs = len(gs)
                    st = psst.tile([128, len(G), 128], F32, tag="st")
                    for gi, g in enumerate(gs):
                        m = qb - g
                        nc.tensor.matmul(
                            st[:, gi, :],
                            lhsT=qkT[pq:pq + 64, 2 + pg, m * 128:(m + 1) * 128],
                            rhs=qkT[pq:pq + 64, pg, qb * 128:(qb + 1) * 128],
                            start=True, stop=True, skip_group_check=True,
                        )
                    expst = dpool.tile([128, len(G), 128], F16, tag="expst")
                    nc.scalar.activation(
                        expst[:, 0:ngs, :], st[:, 0:ngs, :],
                        mybir.ActivationFunctionType.Exp,
                    )
                    nc.vector.tensor_mul(
                        expst[:, 0:ngs, :], expst[:, 0:ngs, :],
                        maskt_sb[:, h, 0:ngs, :],
                    )
                    return expst, gs

                def emit_av(h, qb, av, expst, gs):
                    qs = qb % 4
                    ngs = len(gs)
                    for gi, g in enumerate(gs):
                        m = qb - g
                        nc.tensor.matmul(
                            av[:, qs * 128:(qs + 1) * 128],
                            lhsT=vsb[:, m, 128 * h:128 * h + 128],
                            rhs=expst[:, gi, :],
                            start=(gi == 0), stop=(gi == ngs - 1),
                            skip_group_check=True,
                        )

                normq = []   # deferred norm stage-2 (odd-head lift + gate mul)

                def emit_norm(h, qbg, av, lo=0, hi=512, defer=True):
                    # av rows 0..63 = unnormalized out, row 64 = denominator.
                    # All DVE ops must be partition-aligned: the reciprocal
                    # stays on partition 64, and for odd heads (fgfinal half
                    # 64:128) tmp is lifted 0:64 -> 64:128 with a small
                    # identity matmul before the gate multiply. Stage 2 is
                    # deferred so the PE queue never waits on the chain.
                    w = hi - lo
                    rcp = epool.tile([1, 512], F32, tag="rcp")
                    rbc = epool.tile([128, 512], F32, tag="rbc")
                    tmp = epool.tile([128, 512], F16, tag="tmp")
                    nc.vector.reciprocal_approx_fast(
                        rcp[:, 0:w], av[0:1, lo:hi])
                    nc.gpsimd.partition_broadcast(rbc[:, 0:w], rcp[:, 0:w])
                    nc.vector.tensor_mul(
                        tmp[64:128, 0:w], av[64:128, lo:hi], rbc[64:128, 0:w])
                    normq.append((h, qbg, lo, hi, tmp))
                    if not defer:
                        while normq:
                            norm_stage2()

                def norm_stage2():
                    h, qbg, lo, hi, tmp = normq.pop(0)
                    pq = 64 * (h % 2)
                    pg = h // 2
                    w = hi - lo
                    cols = slice(qbg * 512 + lo, qbg * 512 + hi)
                    if pq == 64:
                        nc.vector.tensor_mul(
                            fgfinal[64:128, pg, cols], tmp[64:128, 0:w],
                            gateT[64:128, pg, cols])
                    else:
                        ps2 = psproj.tile([128, 512], F32, tag="proj")
                        nc.tensor.matmul(
                            ps2[0:64, 0:w], lhsT=idhi_sb[64:128, :],
                            rhs=tmp[64:128, 0:w], start=True, stop=True)
                        nc.vector.tensor_mul(
                            fgfinal[0:64, pg, cols], ps2[0:64, 0:w],
                            gateT[0:64, pg, cols])

                YEV = [nc.vector.tensor_copy, nc.scalar.copy]
                YEV_TAIL = [nc.vector.tensor_copy, nc.scalar.copy]

                def outproj_unit(tci, nt2, tail=False):
                    psy = psproj.tile([128, 512], F32, tag="proj")
                    for kc2 in range(2):
                        nc.tensor.matmul(
                            psy,
                            lhsT=fgfinal[:, kc2, tci * 128:(tci + 1) * 128],
                            rhs=wo_sb[:, kc2, nt2 * 512:(nt2 + 1) * 512],
                            start=(kc2 == 0), stop=(kc2 == 1),
                        )
                    ysb = ypool.tile([128, 512], F16, tag="y")
                    ev = YEV_TAIL if tail else YEV
                    ev[(tci * 2 + nt2) % 2](ysb, psy)
                    nc.sync.dma_start(
                        out=y.ap()[tci * 128:(tci + 1) * 128,
                                   nt2 * 512:(nt2 + 1) * 512],
                        in_=ysb)

                # fully software-pipelined emit: per 512-token chunk nt, run
                # projections for that chunk, then attention for query-block
                # group nt (whose keys only need chunks <= nt), with the
                # score->exp->mask->AV chain skewed one query-block ahead and
                # the previous group's output projection as PE filler.
                SKEW = 3
                HORDER = [0, 2, 1, 3]
                pending = []
                av = None
                outq = []   # pending outproj (tci, nt2) units

                def drain_one():
                    ph, pqb, pexpst, pgs, pav = pending.pop(0)
                    emit_av(ph, pqb, pav, pexpst, pgs)
                    if ph == HORDER[-1] and pqb >= NB - 4:
                        # last head of last round: per-block norm so the
                        # closing dependency chain is short
                        qs = pqb % 4
                        emit_norm(ph, pqb // 4, pav, qs * 128, (qs + 1) * 128,
                                  defer=False)
                        outq.append((pqb, 0))
                        outq.append((pqb, 1))
                    elif pqb % 4 == 3:
                        emit_norm(ph, pqb // 4, pav)
                    if len(normq) > 2:
                        norm_stage2()

                for nt in range(4):
                    if nt == 0:
                        for gi in range(4):
                            do_A(gi, nt)
                        for tci in range(0, 4):
                            do_B(tci)
                    if nt > 0:
                        for tci in range(4 * nt, 4 * nt + 4):
                            do_B(tci)
                    last = nt == 3
                    for h in HORDER:
                        if nt < 3:
                            do_A(HORDER.index(h), nt + 1)   # next round's projection as PE filler
                        if h in (0, 1):
                            do_C(h % 2, nt)
                        for qs in range(4):
                            qb = nt * 4 + qs
                            cur = emit_scores(h, qb)
                            if qs == 0:
                                av = psav.tile([128, 512], F32, tag="av")
                            pending.append((h, qb, cur[0], cur[1], av))
                            if len(pending) > SKEW:
                                drain_one()
                            # outproj units of the previous group, skipping
                            # the first head (norms may not be emitted yet);
                            # flush deferred norm stage-2 before the first pop
                            if h == HORDER[1] and qs == 0:
                                while normq:
                                    norm_stage2()
                            if h != HORDER[0] and outq:
                                outproj_unit(*outq.pop(0))
                    if not last:
                        outq.extend((nt * 4 + i, n2)
                                    for i in range(4) for n2 in range(2))
                while pending:
                    drain_one()
                while normq:
                    norm_stage2()
                while outq:
                    tci, nt2 = outq.pop(0)
                    outproj_unit(tci, nt2, tail=True)
                load.release()

    nc.compile()
    return nc


def make_core_inputs(inputs, b, hg):
    x = np.asarray(inputs["x"], np.float32)
    Wqkv = np.asarray(inputs["Wqkv"], np.float32)
    bqkv = np.asarray(inputs["bqkv"], np.float32)
    Wgate = np.asarray(inputs["Wgate"], np.float32)
    bgate = np.asarray(inputs["bgate"], np.float32)
    Wout = np.asarray(inputs["Wout"], np.float32)
    pos_bias = np.asarray(inputs["pos_bias"], np.float32)

    H0 = HG * hg
    xT = np.ascontiguousarray(x[b].T).reshape(8, 128, N).transpose(1, 0, 2)

    cols = []
    for base in (0, D):   # q then k
        for hp in range(2):
            for hh in range(2):
                hglob = H0 + 2 * hp + hh
                cols.append(np.arange(base + 64 * hglob, base + 64 * hglob + 64))
    cols = np.concatenate(cols)
    wqkm = Wqkv[:, cols].copy()
    wqkm[:, 0:256] *= HD ** -0.5          # fold q scale into weights
    wqk = wqkm.reshape(8, 128, 512).transpose(1, 0, 2)
    bqk = bqkv[cols].copy()
    bqk[0:256] *= HD ** -0.5
    bqk2 = np.ascontiguousarray(bqk.reshape(4, 128).T)

    vcols = np.arange(2 * D + 64 * H0, 2 * D + 64 * H0 + 256)
    wv = Wqkv[:, vcols].reshape(8, 128, 256).transpose(1, 0, 2)
    bvrow = bqkv[vcols].reshape(1, 256)

    gcols = np.arange(256 * hg, 256 * hg + 256)
    f8np = mybir.dt.np(mybir.dt.float8e4)
    xT8 = np.ascontiguousarray(
        x[b].T.reshape(4, 2, 128, N).transpose(2, 0, 1, 3).astype(f8np))
    wg8 = np.ascontiguousarray(
        Wgate[:, gcols].reshape(4, 2, 128, 256).transpose(2, 0, 1, 3).astype(f8np))
    # gate computed as 0.5*tanh((z+b)/2)+0.5 == sigmoid(z+b); activation
    # applies func(in*scale + bias) with scale=0.5, so pre-halve the bias
    bg2 = np.ascontiguousarray((0.5 * bgate[gcols]).reshape(2, 128).T)

    wo = Wout[256 * hg:256 * hg + 256, :].reshape(2, 128, D).transpose(1, 0, 2)

    # exp(pos_bias) at tap positions, 0 elsewhere (multiplied after exp(S))
    off_idx = {d: i for i, d in enumerate(OFFSETS)}
    jj = np.arange(128)[:, None]
    ii = np.arange(128)[None, :]
    maskt = np.zeros((128, HG, len(G), 128), np.float32)
    for gi, g in enumerate(G):
        delta = 128 * g + ii - jj
        sels = [(delta == dlt, oi) for dlt, oi in off_idx.items() if
                -127 <= dlt - 128 * g <= 127]
        for hl in range(HG):
            m = np.zeros((128, 128), np.float32)
            for sel, oi in sels:
                m[sel] = np.exp(pos_bias[oi, H0 + hl])
            maskt[:, hl, gi, :] = m

    idhi = np.zeros((128, 64), np.float32)
    idhi[64:128] = np.eye(64)
    f16c = lambda a: np.ascontiguousarray(a, np.float16)
    return dict(
        xT=f16c(xT), wqk=f16c(wqk), wv=f16c(wv), xT8=xT8, wg8=wg8,
        wo=f16c(wo), maskt=f16c(maskt), bvrow=f16c(bvrow), idhi=f16c(idhi),
        bqk2=bqk2.astype(np.float32), bg2=bg2.astype(np.float32),
    )


_CACHE = {}


def _get_nc():
    if "nc" not in _CACHE:
        _CACHE["nc"] = build_nc()
    return _CACHE["nc"]


def kernel(**inputs):
    nc = _get_nc()
    in_maps = [make_core_inputs(inputs, c // 4, c % 4) for c in range(8)]
    res = run_bass_kernel_spmd(nc, in_maps, core_ids=list(range(8)))
    bout = np.asarray(inputs["bout"], np.float32)
    out = np.zeros((B, N, D), np.float32)
    for c in range(8):
        out[c // 4] += res.results[c]["y"].astype(np.float32)
    out += bout
    return out
